# revision 34
# baseline (speedup 1.0000x reference)
"""Trainium2 Bass kernel: scaled-softmax attention, B=4 H=16 S=2048 D=64.

Sharding: batch*heads (64) across 8 NeuronCores, 8 heads per core.

Per head, on-device, processed in two 1024-query halves (qg):
  for each k-block kb (128 keys):
    S^T[kb] = kT_aug[kb] @ qT_aug    (fp16 x fp16 - same 11-bit mantissa as
              fp32r but keeps the whole matmul stream in 16-bit mode: no
              ~195ns PE dtype-switch penalty, and every matmul counts as HAM
              activity so the 2.4GHz clock never re-throttles. Contraction
              65 = 64 dims + fused row subtracting the per-query bound m_hat)
    P^T[kb] = exp(S^T[kb])           every wave split at the j-chunk
              boundary across both engines working concurrently (frees the
              PSUM wave bank ~500ns earlier -> deeper effective pipeline):
                - ACT (scalar): table exp on cols [0:512]
                - DVE (vector): Schraudolph bit-trick exp on [512:1024] -
                  bf16 bit patterns via round-to-nearest f32->uint16 convert
                  with saturation (deep-negative -> +0.0)
    av[qc] += [v|1][kb] @ P^T[kb]    (fp16 x bf16, K=128 accumulated in PSUM;
              the ones-column makes row 64 the softmax denominator; issued
              one k-block behind the QK/exp front so a late exp never stalls
              the in-order tensor queue)
  outT (rows 0..63 = unnormalized out^T, row 64 = denominator) -> HBM
  (drain copies split ACT/DVE, DMA from the idle gpsimd software-DGE).

DMA-issue plumbing (each dma_start costs ~0.6-1us of issuing-queue time):
sync queue carries only kT[:, :128] + the three qT chunks per head in
first-need order; the otherwise-idle gpsimd issues kT[:, 128:] + v + all
output drains, so the sync queue reaches head 0's transfers immediately and
never backs up behind output traffic.

bf16 warm-up matmuls on const data run before the first real matmul to trip
the PE HAM un-throttle (1.2 -> 2.4 GHz) while the first input DMAs land.

Host (numpy) does input/output marshaling: q scaled by 1/(scale_factor*
inv_scale), m_hat = 5*||q_scaled||, transpose/augment/fp16 rounding on the
way in; per-query divide by the denominator row + transpose on the way out.
"""

import os
import sys

sys.path.insert(0, "/opt/trn_rl_repo")

from contextlib import ExitStack

import numpy as np

import concourse.bass as bass
import concourse.tile as tile
from concourse import bacc, mybir
from concourse.alu_op_type import AluOpType
from concourse.bass_utils import run_bass_kernel_spmd

B, H, S, D = 4, 16, 2048, 64
N_CORES = 8
HPC = (B * H) // N_CORES  # heads per core
KB = S // 128  # 16 k-blocks
DA = D + 1  # augmented contraction dim (65)

F32 = mybir.dt.float32
F32R = mybir.dt.float32r
BF16 = mybir.dt.bfloat16
F16 = mybir.dt.float16
U16 = mybir.dt.uint16

# Schraudolph bf16 exp: bits16 = round(x * EXP_A + EXP_B); rel err ~ +-3.1%
EXP_A = 184.6650390625  # 2^7 / ln(2)
EXP_B = 16250.25  # 127*2^7 with minimax centering

N_WARM = 2  # bf16 HAM warm-up pairs: bridge until the first input DMA lands

LAST_RESULT = None
_CACHED_NC = None


def _maybe_install_ntff_hook():
    """BASS_TRACE=1 needs antenv.axon_hooks, absent from this image; inject it."""
    if not os.environ.get("BASS_TRACE") or "antenv.axon_hooks" in sys.modules:
        return
    try:
        import types

        import antenv
        from trn_agent_boot.trn_boot import _ntff_profile_via_ctypes

        mod = types.ModuleType("antenv.axon_hooks")
        mod._hook = None
        mod.set_axon_ntff_profile_hook = lambda h: setattr(mod, "_hook", h)
        mod.get_axon_ntff_profile_hook = lambda: mod._hook
        sys.modules["antenv.axon_hooks"] = mod
        antenv.axon_hooks = mod
        mod.set_axon_ntff_profile_hook(
            _ntff_profile_via_ctypes("/opt/axon/libaxon_pjrt.so")
        )
    except Exception:
        os.environ["BASS_NEVER_TRACE"] = "1"


def _kt_col(kb):
    """Column of kT block kb inside the packed kq layout."""
    return kb * 128 if kb < 2 else 1280 + (kb - 2) * 128


def _q_col(qc):
    """Column of qT chunk qc (512 queries) inside the packed kq layout."""
    return 256 + qc * 512 if qc < 2 else 3072 + (qc - 2) * 512


def _build_nc():
    nc = bacc.Bacc("TRN2", target_bir_lowering=False, debug=False)

    # kq packs kT and qT per head in first-need column order:
    #   [0:256]=kT blk0-1 | [256:1280]=qT chunk0-1 | [1280:3072]=kT blk2-15
    #   | [3072:4096]=qT chunk2-3
    # so head 0 streams in as two contiguous pieces (piece A = cols 0:1280
    # covers the whole first ~2us of compute) and later heads as one DMA —
    # each dma_start costs ~0.8us of issuing-queue time, so fewer, bigger
    # issues in need order is what gets the PE fed early.
    d_kq = nc.dram_tensor("kq", [HPC, DA, 2 * S], F16, kind="ExternalInput").ap()
    d_v = nc.dram_tensor("v", [HPC, 128, KB, DA], F16, kind="ExternalInput").ap()
    d_out = nc.dram_tensor("outT", [HPC, DA, S], F32, kind="ExternalOutput").ap()

    with tile.TileContext(nc) as tc, ExitStack() as ctx:
        cpool = ctx.enter_context(tc.tile_pool(name="consts", bufs=1))
        inpool = ctx.enter_context(tc.tile_pool(name="in", bufs=3))
        ptpool = ctx.enter_context(tc.tile_pool(name="pt", bufs=4))
        wkpool = ctx.enter_context(tc.tile_pool(name="wk", bufs=3))
        qkp = ctx.enter_context(tc.tile_pool(name="qkp", bufs=3, space="PSUM"))
        mp = ctx.enter_context(tc.tile_pool(name="mp", bufs=1, space="PSUM"))

        # warm-up consts: memsets on DVE, whose queue comes up first and has
        # nothing else to do, so the HAM warm-up matmuls can issue the moment
        # the tensor queue starts
        t_junkw = cpool.tile([128, 128], BF16)
        nc.vector.memset(t_junkw[:], 0.0)
        t_junk = cpool.tile([128, 512], BF16)
        nc.vector.memset(t_junk[:], 0.0)
        t_warm = cpool.tile([1, 1], F32)
        # trigger the ACT exp table load while input DMAs run
        nc.scalar.activation(
            t_warm[:], t_junkw[0:1, 0:1], mybir.ActivationFunctionType.Exp
        )

        def load_head(h):
            t_kq = inpool.tile([DA, 2 * S], F16, tag="kq", name=f"kq{h}")
            t_v = inpool.tile([128, KB, DA], F16, tag="v", name=f"v{h}")
            if h == 0:
                # head 0's two pieces generate descriptors on the scalar and
                # sync HWDGE queues CONCURRENTLY (~0.8us each); piece A
                # covers the first ~2us of compute
                nc.scalar.dma_start(out=t_kq[:, 0:1280], in_=d_kq[h][:, 0:1280])
                nc.sync.dma_start(out=t_kq[:, 1280:4096], in_=d_kq[h][:, 1280:4096])
            else:
                nc.sync.dma_start(out=t_kq[:], in_=d_kq[h])
            nc.gpsimd.dma_start(out=t_v[:], in_=d_v[h])
            return t_kq, t_v

        # head-0 input DMAs lead the sync queue so the rings start
        # immediately; later heads are requested only after head 0's body is
        # emitted so their bulk transfers cannot crowd head 0's
        # latency-critical pieces off the DMA rings
        pending = [load_head(0)]

        # HAM pre-warm: bf16 matmuls on const data keep the PE busy while the
        # first input DMA lands so the clock un-throttles before real work
        # (fp32-mode matmuls do not register as HAM activity). Two tiny
        # bootstrap matmuls gated only on the small memset start the
        # activity clock ~0.8us earlier; then one tile with alternating
        # halves, WAW-chained back-to-back with no pool rotation gaps.
        pwarm = qkp.tile([128, 1024], F32, tag="wave", name="pwarm")
        for w in range(2):
            nc.tensor.matmul(
                pwarm[:, w * 128 : (w + 1) * 128],
                t_junkw[:],
                t_junkw[:],
                start=True,
                stop=True,
            )
        for w in range(2 * N_WARM):
            nc.tensor.matmul(
                pwarm[:, (w % 2) * 512 : (w % 2 + 1) * 512],
                t_junkw[:],
                t_junk[:],
                start=True,
                stop=True,
            )

        for h in range(HPC):
            t_kq, t_v = pending[h]

            # two 1024-query halves per head: AV accumulators use only 2 PSUM
            # banks so the QK wave pool can run 3-deep (ACT and DVE exps for
            # consecutive k-blocks overlap instead of serializing)
            for qg in range(2):
                p_av = [
                    mp.tile([DA, 512], F32, tag=f"av{j}", name=f"av{h}_{qg}_{j}")
                    for j in range(2)
                ]

                def emit_av(kb, t_pt):
                    for j in range(2):
                        nc.tensor.matmul(
                            p_av[j][:],
                            t_v[:, kb, :],
                            t_pt[:, j * 512 : (j + 1) * 512],
                            start=(kb == 0),
                            stop=(kb == KB - 1),
                        )

                # software pipeline, 1 k-block deep: AV(kb-1) issues after
                # QK(kb)+exp(kb), so each exp gets a full QK-pair + AV-pair
                # (~850ns) of PE time before its dependent AV can stall the
                # in-order tensor queue (2-deep measured worse: two trailing
                # AV pairs per sweep run past the last QK and stall on exps)
                hist = []
                for kb in range(KB):
                    pw = qkp.tile([128, 1024], F32, tag="wave")
                    for j in range(2):
                        qc = qg * 2 + j
                        nc.tensor.matmul(
                            pw[:, j * 512 : (j + 1) * 512],
                            t_kq[:, _kt_col(kb) : _kt_col(kb) + 128],
                            t_kq[:, _q_col(qc) : _q_col(qc) + 512],
                            start=True,
                            stop=True,
                        )
                    t_pt = ptpool.tile(
                        [128, 1024], BF16, tag="pt", name=f"pt{h}_{qg}_{kb}"
                    )
                    # every exp splits at the j-chunk boundary: ACT and
                    # DVE work the two halves concurrently, freeing the
                    # wave bank ~500ns earlier (deeper effective pipeline)
                    nc.scalar.activation(
                        t_pt[:, 0:512],
                        pw[:, 0:512],
                        mybir.ActivationFunctionType.Exp,
                        bias=0.0,
                        scale=1.0,
                    )
                    nc.vector.tensor_scalar(
                        t_pt[:, 512:1024].bitcast(U16),
                        pw[:, 512:1024],
                        EXP_A,
                        EXP_B,
                        AluOpType.mult,
                        AluOpType.add,
                    )
                    hist.append((kb, t_pt))
                    if len(hist) > 1:
                        emit_av(*hist.pop(0))
                for it in hist:
                    emit_av(*it)

                # drain accumulators: outT rows 0..63 = unnormalized out^T,
                # row 64 = softmax denominator; host divides + transposes.
                # Copies split ACT/DVE; the DMA issues from the idle gpsimd
                # queue. The very last drain is latency-critical (nothing
                # left to overlap with) so each 256-col quarter is copied on
                # its own engine and the two DMAs issue concurrently from
                # the by-then-idle sync and scalar HWDGE queues.
                t_outT = wkpool.tile([DA, 1024], F32, tag="outT")
                last = h == HPC - 1 and qg == 1
                if last:
                    nc.scalar.activation(
                        t_outT[:, 0:256],
                        p_av[0][:, 0:256],
                        mybir.ActivationFunctionType.Copy,
                    )
                    nc.vector.tensor_copy(t_outT[:, 256:512], p_av[0][:, 256:512])
                    nc.sync.dma_start(
                        out=d_out[h][:, qg * 1024 : qg * 1024 + 512],
                        in_=t_outT[:, 0:512],
                    )
                    nc.scalar.activation(
                        t_outT[:, 512:768],
                        p_av[1][:, 0:256],
                        mybir.ActivationFunctionType.Copy,
                    )
                    nc.vector.tensor_copy(t_outT[:, 768:1024], p_av[1][:, 256:512])
                    # j1 DMA also on sync: its DIRECT2D runs while the j1
                    # copies finish (gpsimd dispatches ~0.8us late here and
                    # scalar's HWDGE measured ~1.4us)
                    nc.sync.dma_start(
                        out=d_out[h][:, qg * 1024 + 512 : qg * 1024 + 1024],
                        in_=t_outT[:, 512:1024],
                    )
                else:
                    nc.scalar.activation(
                        t_outT[:, 0:512],
                        p_av[0][:],
                        mybir.ActivationFunctionType.Copy,
                    )
                    nc.vector.tensor_copy(t_outT[:, 512:1024], p_av[1][:])
                    nc.gpsimd.dma_start(
                        out=d_out[h][:, qg * 1024 : (qg + 1) * 1024],
                        in_=t_outT[:],
                    )

            # prefetch after the drains so output DMAs never queue behind
            # this load's buffer-free wait on the gpsimd queue
            while len(pending) < min(HPC, h + 3):
                pending.append(load_head(len(pending)))

    nc.compile()
    return nc


def kernel(
    q: np.ndarray,
    k: np.ndarray,
    v: np.ndarray,
    scale_factor: np.ndarray,
    inv_scale: np.ndarray,
) -> np.ndarray:
    global LAST_RESULT, _CACHED_NC

    q = np.asarray(q, np.float32)
    k = np.asarray(k, np.float32)
    v = np.asarray(v, np.float32)
    scale_factor = np.asarray(scale_factor, np.float32)
    inv_scale = np.asarray(inv_scale, np.float32)

    # host-side input marshaling
    r = 1.0 / (scale_factor * inv_scale[..., None])  # [B,H,S]
    qs = q * r[..., None]  # [B,H,S,D]
    mhat = 5.0 * np.sqrt((qs.astype(np.float64) ** 2).sum(-1)).astype(np.float32)
    q_aug = np.concatenate([qs, -mhat[..., None]], axis=-1)  # [B,H,S,DA]
    k_aug = np.concatenate([k, np.ones((B, H, S, 1), np.float32)], axis=-1)
    v_aug = np.concatenate([v, np.ones((B, H, S, 1), np.float32)], axis=-1)

    qT = q_aug.transpose(0, 1, 3, 2).astype(np.float16)  # [B,H,DA,S]
    kT = k_aug.transpose(0, 1, 3, 2).astype(np.float16)
    # packed first-need layout (see _build_nc): kT blk0-1 | qT c0-1 |
    # kT blk2-15 | qT c2-3
    kq = np.concatenate(
        [kT[..., 0:256], qT[..., 0:1024], kT[..., 256:2048], qT[..., 1024:2048]],
        axis=-1,
    )
    # [B,H,S,DA] -> [B,H,KB,128,DA] -> [B,H,128,KB,DA]
    v16 = np.ascontiguousarray(
        v_aug.reshape(B, H, KB, 128, DA).transpose(0, 1, 3, 2, 4)
    ).astype(np.float16)

    kq = np.ascontiguousarray(kq).reshape(N_CORES, HPC, DA, 2 * S)
    v16 = v16.reshape(N_CORES, HPC, 128, KB, DA)

    _maybe_install_ntff_hook()
    if _CACHED_NC is None:
        _CACHED_NC = _build_nc()
    nc = _CACHED_NC

    in_maps = [{"kq": kq[c], "v": v16[c]} for c in range(N_CORES)]
    res = run_bass_kernel_spmd(nc, in_maps, list(range(N_CORES)))
    LAST_RESULT = res
    outT = np.stack([res.results[c]["outT"] for c in range(N_CORES)])  # [8,HPC,DA,S]
    out = outT[:, :, :D, :] / outT[:, :, D : D + 1, :]
    return (
        np.ascontiguousarray(out.transpose(0, 1, 3, 2))
        .reshape(B, H, S, D)
        .astype(np.float32)
    )


# revision 35
# speedup vs baseline: 1.0015x; 1.0015x over previous
"""Trainium2 Bass kernel: scaled-softmax attention, B=4 H=16 S=2048 D=64.

Sharding: batch*heads (64) across 8 NeuronCores, 8 heads per core.

Per head, on-device, processed in two 1024-query halves (qg):
  for each k-block kb (128 keys):
    S^T[kb] = kT_aug[kb] @ qT_aug    (fp16 x fp16 - same 11-bit mantissa as
              fp32r but keeps the whole matmul stream in 16-bit mode: no
              ~195ns PE dtype-switch penalty, and every matmul counts as HAM
              activity so the 2.4GHz clock never re-throttles. Contraction
              65 = 64 dims + fused row subtracting the per-query bound m_hat)
    P^T[kb] = exp(S^T[kb])           every wave split at the j-chunk
              boundary across both engines working concurrently (frees the
              PSUM wave bank ~500ns earlier -> deeper effective pipeline):
                - ACT (scalar): table exp on cols [0:512]
                - DVE (vector): Schraudolph bit-trick exp on [512:1024] -
                  bf16 bit patterns via round-to-nearest f32->uint16 convert
                  with saturation (deep-negative -> +0.0)
    av[qc] += [v|1][kb] @ P^T[kb]    (fp16 x bf16, K=128 accumulated in PSUM;
              the ones-column makes row 64 the softmax denominator; issued
              one k-block behind the QK/exp front so a late exp never stalls
              the in-order tensor queue)
  outT (rows 0..63 = unnormalized out^T, row 64 = denominator) -> HBM
  (drain copies split ACT/DVE, DMA from the idle gpsimd software-DGE).

DMA-issue plumbing (each dma_start costs ~0.6-1us of issuing-queue time):
sync queue carries only kT[:, :128] + the three qT chunks per head in
first-need order; the otherwise-idle gpsimd issues kT[:, 128:] + v + all
output drains, so the sync queue reaches head 0's transfers immediately and
never backs up behind output traffic.

bf16 warm-up matmuls on const data run before the first real matmul to trip
the PE HAM un-throttle (1.2 -> 2.4 GHz) while the first input DMAs land.

Host (numpy) does input/output marshaling: q scaled by 1/(scale_factor*
inv_scale), m_hat = 5*||q_scaled||, transpose/augment/fp16 rounding on the
way in; per-query divide by the denominator row + transpose on the way out.
"""

import os
import sys

sys.path.insert(0, "/opt/trn_rl_repo")

from contextlib import ExitStack

import numpy as np

import concourse.bass as bass
import concourse.tile as tile
from concourse import bacc, mybir
from concourse.alu_op_type import AluOpType
from concourse.bass_utils import run_bass_kernel_spmd

B, H, S, D = 4, 16, 2048, 64
N_CORES = 8
HPC = (B * H) // N_CORES  # heads per core
KB = S // 128  # 16 k-blocks
DA = D + 1  # augmented contraction dim (65)

F32 = mybir.dt.float32
F32R = mybir.dt.float32r
BF16 = mybir.dt.bfloat16
F16 = mybir.dt.float16
U16 = mybir.dt.uint16

# Schraudolph bf16 exp: bits16 = round(x * EXP_A + EXP_B); rel err ~ +-3.1%
EXP_A = 184.6650390625  # 2^7 / ln(2)
EXP_B = 16250.25  # 127*2^7 with minimax centering

N_WARM = 2  # bf16 HAM warm-up pairs: bridge until the first input DMA lands

LAST_RESULT = None
_CACHED_NC = None


def _maybe_install_ntff_hook():
    """BASS_TRACE=1 needs antenv.axon_hooks, absent from this image; inject it."""
    if not os.environ.get("BASS_TRACE") or "antenv.axon_hooks" in sys.modules:
        return
    try:
        import types

        import antenv
        from trn_agent_boot.trn_boot import _ntff_profile_via_ctypes

        mod = types.ModuleType("antenv.axon_hooks")
        mod._hook = None
        mod.set_axon_ntff_profile_hook = lambda h: setattr(mod, "_hook", h)
        mod.get_axon_ntff_profile_hook = lambda: mod._hook
        sys.modules["antenv.axon_hooks"] = mod
        antenv.axon_hooks = mod
        mod.set_axon_ntff_profile_hook(
            _ntff_profile_via_ctypes("/opt/axon/libaxon_pjrt.so")
        )
    except Exception:
        os.environ["BASS_NEVER_TRACE"] = "1"


def _kt_col(kb):
    """Column of kT block kb inside the packed kq layout."""
    return kb * 128 if kb < 2 else 1280 + (kb - 2) * 128


def _q_col(qc):
    """Column of qT chunk qc (512 queries) inside the packed kq layout."""
    return 256 + qc * 512 if qc < 2 else 3072 + (qc - 2) * 512


def _build_nc():
    nc = bacc.Bacc("TRN2", target_bir_lowering=False, debug=False)

    # kq packs kT and qT per head in first-need column order:
    #   [0:256]=kT blk0-1 | [256:1280]=qT chunk0-1 | [1280:3072]=kT blk2-15
    #   | [3072:4096]=qT chunk2-3
    # so head 0 streams in as two contiguous pieces (piece A = cols 0:1280
    # covers the whole first ~2us of compute) and later heads as one DMA —
    # each dma_start costs ~0.8us of issuing-queue time, so fewer, bigger
    # issues in need order is what gets the PE fed early.
    d_kq = nc.dram_tensor("kq", [HPC, DA, 2 * S], F16, kind="ExternalInput").ap()
    d_v = nc.dram_tensor("v", [HPC, 128, KB, DA], F16, kind="ExternalInput").ap()
    d_out = nc.dram_tensor("outT", [HPC, DA, S], F32, kind="ExternalOutput").ap()

    with tile.TileContext(nc) as tc, ExitStack() as ctx:
        cpool = ctx.enter_context(tc.tile_pool(name="consts", bufs=1))
        inpool = ctx.enter_context(tc.tile_pool(name="in", bufs=3))
        ptpool = ctx.enter_context(tc.tile_pool(name="pt", bufs=6))
        wkpool = ctx.enter_context(tc.tile_pool(name="wk", bufs=4))
        qkp = ctx.enter_context(tc.tile_pool(name="qkp", bufs=3, space="PSUM"))
        mp = ctx.enter_context(tc.tile_pool(name="mp", bufs=1, space="PSUM"))

        # warm-up consts: memsets on DVE, whose queue comes up first and has
        # nothing else to do, so the HAM warm-up matmuls can issue the moment
        # the tensor queue starts
        t_junkw = cpool.tile([128, 128], BF16)
        nc.vector.memset(t_junkw[:], 0.0)
        t_junk = cpool.tile([128, 512], BF16)
        nc.vector.memset(t_junk[:], 0.0)
        t_warm = cpool.tile([1, 1], F32)
        # trigger the ACT exp table load while input DMAs run
        nc.scalar.activation(
            t_warm[:], t_junkw[0:1, 0:1], mybir.ActivationFunctionType.Exp
        )

        def load_head(h):
            t_kq = inpool.tile([DA, 2 * S], F16, tag="kq", name=f"kq{h}")
            t_v = inpool.tile([128, KB, DA], F16, tag="v", name=f"v{h}")
            if h == 0:
                # split so the first piece (everything the first ~2us of
                # compute touches) lands as early as possible
                nc.sync.dma_start(out=t_kq[:, 0:1280], in_=d_kq[h][:, 0:1280])
                nc.sync.dma_start(out=t_kq[:, 1280:4096], in_=d_kq[h][:, 1280:4096])
            else:
                nc.sync.dma_start(out=t_kq[:], in_=d_kq[h])
            nc.gpsimd.dma_start(out=t_v[:], in_=d_v[h])
            return t_kq, t_v

        # head-0 input DMAs lead the sync queue so the rings start
        # immediately; later heads are requested only after head 0's body is
        # emitted so their bulk transfers cannot crowd head 0's
        # latency-critical pieces off the DMA rings
        pending = [load_head(0)]

        # HAM pre-warm: bf16 matmuls on const data keep the PE busy while the
        # first input DMA lands so the clock un-throttles before real work
        # (fp32-mode matmuls do not register as HAM activity). Two tiny
        # bootstrap matmuls gated only on the small memset start the
        # activity clock ~0.8us earlier; then one tile with alternating
        # halves, WAW-chained back-to-back with no pool rotation gaps.
        pwarm = qkp.tile([128, 1024], F32, tag="wave", name="pwarm")
        for w in range(2):
            nc.tensor.matmul(
                pwarm[:, w * 128 : (w + 1) * 128],
                t_junkw[:],
                t_junkw[:],
                start=True,
                stop=True,
            )
        for w in range(2 * N_WARM):
            nc.tensor.matmul(
                pwarm[:, (w % 2) * 512 : (w % 2 + 1) * 512],
                t_junkw[:],
                t_junk[:],
                start=True,
                stop=True,
            )

        for h in range(HPC):
            t_kq, t_v = pending[h]

            # two 1024-query halves per head: AV accumulators use only 2 PSUM
            # banks so the QK wave pool can run 3-deep (ACT and DVE exps for
            # consecutive k-blocks overlap instead of serializing)
            for qg in range(2):
                p_av = [
                    mp.tile([DA, 512], F32, tag=f"av{j}", name=f"av{h}_{qg}_{j}")
                    for j in range(2)
                ]

                def emit_av(kb, t_pt):
                    for j in range(2):
                        nc.tensor.matmul(
                            p_av[j][:],
                            t_v[:, kb, :],
                            t_pt[:, j * 512 : (j + 1) * 512],
                            start=(kb == 0),
                            stop=(kb == KB - 1),
                        )

                # software pipeline, 1 k-block deep: AV(kb-1) issues after
                # QK(kb)+exp(kb), so each exp gets a full QK-pair + AV-pair
                # (~850ns) of PE time before its dependent AV can stall the
                # in-order tensor queue (2-deep measured worse: two trailing
                # AV pairs per sweep run past the last QK and stall on exps)
                hist = []
                for kb in range(KB):
                    pw = qkp.tile([128, 1024], F32, tag="wave")
                    for j in range(2):
                        qc = qg * 2 + j
                        nc.tensor.matmul(
                            pw[:, j * 512 : (j + 1) * 512],
                            t_kq[:, _kt_col(kb) : _kt_col(kb) + 128],
                            t_kq[:, _q_col(qc) : _q_col(qc) + 512],
                            start=True,
                            stop=True,
                        )
                    t_pt = ptpool.tile(
                        [128, 1024], BF16, tag="pt", name=f"pt{h}_{qg}_{kb}"
                    )
                    # every exp splits at the j-chunk boundary: ACT and
                    # DVE work the two halves concurrently, freeing the
                    # wave bank ~500ns earlier (deeper effective pipeline)
                    nc.scalar.activation(
                        t_pt[:, 0:512],
                        pw[:, 0:512],
                        mybir.ActivationFunctionType.Exp,
                        bias=0.0,
                        scale=1.0,
                    )
                    nc.vector.tensor_scalar(
                        t_pt[:, 512:1024].bitcast(U16),
                        pw[:, 512:1024],
                        EXP_A,
                        EXP_B,
                        AluOpType.mult,
                        AluOpType.add,
                    )
                    hist.append((kb, t_pt))
                    if len(hist) > 1:
                        emit_av(*hist.pop(0))
                for it in hist:
                    emit_av(*it)

                # drain accumulators: outT rows 0..63 = unnormalized out^T,
                # row 64 = softmax denominator; host divides + transposes.
                # Copies split ACT/DVE; the DMA issues from the idle gpsimd
                # queue. The very last drain is latency-critical (nothing
                # left to overlap with) so each 256-col quarter is copied on
                # its own engine and the two DMAs issue concurrently from
                # the by-then-idle sync and scalar HWDGE queues.
                t_outT = wkpool.tile([DA, 1024], F32, tag="outT")
                last = h == HPC - 1 and qg == 1
                if last:
                    nc.scalar.activation(
                        t_outT[:, 0:256],
                        p_av[0][:, 0:256],
                        mybir.ActivationFunctionType.Copy,
                    )
                    nc.vector.tensor_copy(t_outT[:, 256:512], p_av[0][:, 256:512])
                    nc.sync.dma_start(
                        out=d_out[h][:, qg * 1024 : qg * 1024 + 512],
                        in_=t_outT[:, 0:512],
                    )
                    nc.scalar.activation(
                        t_outT[:, 512:768],
                        p_av[1][:, 0:256],
                        mybir.ActivationFunctionType.Copy,
                    )
                    nc.vector.tensor_copy(t_outT[:, 768:1024], p_av[1][:, 256:512])
                    # j1 DMA also on sync: its DIRECT2D runs while the j1
                    # copies finish (gpsimd dispatches ~0.8us late here and
                    # scalar's HWDGE measured ~1.4us)
                    nc.sync.dma_start(
                        out=d_out[h][:, qg * 1024 + 512 : qg * 1024 + 1024],
                        in_=t_outT[:, 512:1024],
                    )
                else:
                    nc.scalar.activation(
                        t_outT[:, 0:512],
                        p_av[0][:],
                        mybir.ActivationFunctionType.Copy,
                    )
                    nc.vector.tensor_copy(t_outT[:, 512:1024], p_av[1][:])
                    nc.gpsimd.dma_start(
                        out=d_out[h][:, qg * 1024 : (qg + 1) * 1024],
                        in_=t_outT[:],
                    )

            # prefetch after the drains so output DMAs never queue behind
            # this load's buffer-free wait on the gpsimd queue
            while len(pending) < min(HPC, h + 3):
                pending.append(load_head(len(pending)))

    nc.compile()
    return nc


def kernel(
    q: np.ndarray,
    k: np.ndarray,
    v: np.ndarray,
    scale_factor: np.ndarray,
    inv_scale: np.ndarray,
) -> np.ndarray:
    global LAST_RESULT, _CACHED_NC

    q = np.asarray(q, np.float32)
    k = np.asarray(k, np.float32)
    v = np.asarray(v, np.float32)
    scale_factor = np.asarray(scale_factor, np.float32)
    inv_scale = np.asarray(inv_scale, np.float32)

    # host-side input marshaling
    r = 1.0 / (scale_factor * inv_scale[..., None])  # [B,H,S]
    qs = q * r[..., None]  # [B,H,S,D]
    mhat = 5.0 * np.sqrt((qs.astype(np.float64) ** 2).sum(-1)).astype(np.float32)
    q_aug = np.concatenate([qs, -mhat[..., None]], axis=-1)  # [B,H,S,DA]
    k_aug = np.concatenate([k, np.ones((B, H, S, 1), np.float32)], axis=-1)
    v_aug = np.concatenate([v, np.ones((B, H, S, 1), np.float32)], axis=-1)

    qT = q_aug.transpose(0, 1, 3, 2).astype(np.float16)  # [B,H,DA,S]
    kT = k_aug.transpose(0, 1, 3, 2).astype(np.float16)
    # packed first-need layout (see _build_nc): kT blk0-1 | qT c0-1 |
    # kT blk2-15 | qT c2-3
    kq = np.concatenate(
        [kT[..., 0:256], qT[..., 0:1024], kT[..., 256:2048], qT[..., 1024:2048]],
        axis=-1,
    )
    # [B,H,S,DA] -> [B,H,KB,128,DA] -> [B,H,128,KB,DA]
    v16 = np.ascontiguousarray(
        v_aug.reshape(B, H, KB, 128, DA).transpose(0, 1, 3, 2, 4)
    ).astype(np.float16)

    kq = np.ascontiguousarray(kq).reshape(N_CORES, HPC, DA, 2 * S)
    v16 = v16.reshape(N_CORES, HPC, 128, KB, DA)

    _maybe_install_ntff_hook()
    if _CACHED_NC is None:
        _CACHED_NC = _build_nc()
    nc = _CACHED_NC

    in_maps = [{"kq": kq[c], "v": v16[c]} for c in range(N_CORES)]
    res = run_bass_kernel_spmd(nc, in_maps, list(range(N_CORES)))
    LAST_RESULT = res
    outT = np.stack([res.results[c]["outT"] for c in range(N_CORES)])  # [8,HPC,DA,S]
    out = outT[:, :, :D, :] / outT[:, :, D : D + 1, :]
    return (
        np.ascontiguousarray(out.transpose(0, 1, 3, 2))
        .reshape(B, H, S, D)
        .astype(np.float32)
    )


# revision 37
# speedup vs baseline: 1.0017x; 1.0003x over previous
"""Trainium2 Bass kernel: scaled-softmax attention, B=4 H=16 S=2048 D=64.

Sharding: batch*heads (64) across 8 NeuronCores, 8 heads per core.

Per head, on-device, processed in two 1024-query halves (qg):
  for each k-block kb (128 keys):
    S^T[kb] = kT_aug[kb] @ qT_aug    (fp16 x fp16 - same 11-bit mantissa as
              fp32r but keeps the whole matmul stream in 16-bit mode: no
              ~195ns PE dtype-switch penalty, and every matmul counts as HAM
              activity so the 2.4GHz clock never re-throttles. Contraction
              65 = 64 dims + fused row subtracting the per-query bound m_hat)
    P^T[kb] = exp(S^T[kb])           every wave split at the j-chunk
              boundary across both engines working concurrently (frees the
              PSUM wave bank ~500ns earlier -> deeper effective pipeline):
                - ACT (scalar): table exp on cols [0:512]
                - DVE (vector): Schraudolph bit-trick exp on [512:1024] -
                  bf16 bit patterns via round-to-nearest f32->uint16 convert
                  with saturation (deep-negative -> +0.0)
    av[qc] += [v|1][kb] @ P^T[kb]    (fp16 x bf16, K=128 accumulated in PSUM;
              the ones-column makes row 64 the softmax denominator; issued
              one k-block behind the QK/exp front so a late exp never stalls
              the in-order tensor queue)
  outT (rows 0..63 = unnormalized out^T, row 64 = denominator) -> HBM
  (drain copies split ACT/DVE, DMA from the idle gpsimd software-DGE).

DMA-issue plumbing (each dma_start costs ~0.6-1us of issuing-queue time):
kT and qT are packed host-side into one dram tensor per head in first-need
column order, so each head is a single sync-queue DMA (head 0: two pieces,
the first covering the initial ~2us of compute); the otherwise-idle gpsimd
software-DGE issues v + all output drains so the sync queue reaches head
0's transfers immediately and never backs up behind output traffic. Later
heads prefetch only after the current head's body so their bulk transfers
cannot crowd the latency-critical first pieces off the DMA rings.

bf16 warm-up matmuls on const data run before the first real matmul to trip
the PE HAM un-throttle (1.2 -> 2.4 GHz) while the first input DMAs land.

Host (numpy) does input/output marshaling: q scaled by 1/(scale_factor*
inv_scale), m_hat = 5*||q_scaled||, transpose/augment/fp16 rounding on the
way in; per-query divide by the denominator row + transpose on the way out.
"""

import os
import sys

sys.path.insert(0, "/opt/trn_rl_repo")

from contextlib import ExitStack

import numpy as np

import concourse.bass as bass
import concourse.tile as tile
from concourse import bacc, mybir
from concourse.alu_op_type import AluOpType
from concourse.bass_utils import run_bass_kernel_spmd

B, H, S, D = 4, 16, 2048, 64
N_CORES = 8
HPC = (B * H) // N_CORES  # heads per core
KB = S // 128  # 16 k-blocks
DA = D + 1  # augmented contraction dim (65)

F32 = mybir.dt.float32
F32R = mybir.dt.float32r
BF16 = mybir.dt.bfloat16
F16 = mybir.dt.float16
U16 = mybir.dt.uint16

# Schraudolph bf16 exp: bits16 = round(x * EXP_A + EXP_B); rel err ~ +-3.1%
EXP_A = 184.6650390625  # 2^7 / ln(2)
EXP_B = 16250.25  # 127*2^7 with minimax centering

N_WARM = 2  # bf16 HAM warm-up pairs: bridge until the first input DMA lands

LAST_RESULT = None
_CACHED_NC = None


def _maybe_install_ntff_hook():
    """BASS_TRACE=1 needs antenv.axon_hooks, absent from this image; inject it."""
    if not os.environ.get("BASS_TRACE") or "antenv.axon_hooks" in sys.modules:
        return
    try:
        import types

        import antenv
        from trn_agent_boot.trn_boot import _ntff_profile_via_ctypes

        mod = types.ModuleType("antenv.axon_hooks")
        mod._hook = None
        mod.set_axon_ntff_profile_hook = lambda h: setattr(mod, "_hook", h)
        mod.get_axon_ntff_profile_hook = lambda: mod._hook
        sys.modules["antenv.axon_hooks"] = mod
        antenv.axon_hooks = mod
        mod.set_axon_ntff_profile_hook(
            _ntff_profile_via_ctypes("/opt/axon/libaxon_pjrt.so")
        )
    except Exception:
        os.environ["BASS_NEVER_TRACE"] = "1"


def _kt_col(kb):
    """Column of kT block kb inside the packed kq layout."""
    return kb * 128 if kb < 2 else 1280 + (kb - 2) * 128


def _q_col(qc):
    """Column of qT chunk qc (512 queries) inside the packed kq layout."""
    return 256 + qc * 512 if qc < 2 else 3072 + (qc - 2) * 512


def _build_nc():
    nc = bacc.Bacc("TRN2", target_bir_lowering=False, debug=False)

    # kq packs kT and qT per head in first-need column order:
    #   [0:256]=kT blk0-1 | [256:1280]=qT chunk0-1 | [1280:3072]=kT blk2-15
    #   | [3072:4096]=qT chunk2-3
    # so head 0 streams in as two contiguous pieces (piece A = cols 0:1280
    # covers the whole first ~2us of compute) and later heads as one DMA —
    # each dma_start costs ~0.8us of issuing-queue time, so fewer, bigger
    # issues in need order is what gets the PE fed early.
    d_kq = nc.dram_tensor("kq", [HPC, DA, 2 * S], F16, kind="ExternalInput").ap()
    d_v = nc.dram_tensor("v", [HPC, 128, KB, DA], F16, kind="ExternalInput").ap()
    d_out = nc.dram_tensor("outT", [HPC, DA, S], F32, kind="ExternalOutput").ap()

    with tile.TileContext(nc) as tc, ExitStack() as ctx:
        cpool = ctx.enter_context(tc.tile_pool(name="consts", bufs=1))
        inpool = ctx.enter_context(tc.tile_pool(name="in", bufs=3))
        ptpool = ctx.enter_context(tc.tile_pool(name="pt", bufs=4))
        wkpool = ctx.enter_context(tc.tile_pool(name="wk", bufs=3))
        qkp = ctx.enter_context(tc.tile_pool(name="qkp", bufs=3, space="PSUM"))
        mp = ctx.enter_context(tc.tile_pool(name="mp", bufs=1, space="PSUM"))

        # warm-up consts: memsets on DVE, whose queue comes up first and has
        # nothing else to do, so the HAM warm-up matmuls can issue the moment
        # the tensor queue starts
        t_junkw = cpool.tile([128, 128], BF16)
        nc.vector.memset(t_junkw[:], 0.0)
        t_junk = cpool.tile([128, 512], BF16)
        nc.vector.memset(t_junk[:], 0.0)
        t_warm = cpool.tile([1, 1], F32)
        # trigger the ACT exp table load while input DMAs run
        nc.scalar.activation(
            t_warm[:], t_junkw[0:1, 0:1], mybir.ActivationFunctionType.Exp
        )

        def load_head(h):
            t_kq = inpool.tile([DA, 2 * S], F16, tag="kq", name=f"kq{h}")
            t_v = inpool.tile([128, KB, DA], F16, tag="v", name=f"v{h}")
            if h == 0:
                # split so the first piece (everything the first ~2us of
                # compute touches) lands as early as possible
                nc.sync.dma_start(out=t_kq[:, 0:1280], in_=d_kq[h][:, 0:1280])
                nc.sync.dma_start(out=t_kq[:, 1280:4096], in_=d_kq[h][:, 1280:4096])
            else:
                nc.sync.dma_start(out=t_kq[:], in_=d_kq[h])
            nc.gpsimd.dma_start(out=t_v[:], in_=d_v[h])
            return t_kq, t_v

        # head-0 input DMAs lead the sync queue so the rings start
        # immediately; later heads are requested only after head 0's body is
        # emitted so their bulk transfers cannot crowd head 0's
        # latency-critical pieces off the DMA rings
        pending = [load_head(0)]

        # HAM pre-warm: bf16 matmuls on const data keep the PE busy while the
        # first input DMA lands so the clock un-throttles before real work
        # (fp32-mode matmuls do not register as HAM activity). Two tiny
        # bootstrap matmuls gated only on the small memset start the
        # activity clock ~0.8us earlier; then one tile with alternating
        # halves, WAW-chained back-to-back with no pool rotation gaps.
        pwarm = qkp.tile([128, 1024], F32, tag="wave", name="pwarm")
        for w in range(2):
            nc.tensor.matmul(
                pwarm[:, w * 128 : (w + 1) * 128],
                t_junkw[:],
                t_junkw[:],
                start=True,
                stop=True,
            )
        for w in range(2 * N_WARM):
            nc.tensor.matmul(
                pwarm[:, (w % 2) * 512 : (w % 2 + 1) * 512],
                t_junkw[:],
                t_junk[:],
                start=True,
                stop=True,
            )

        for h in range(HPC):
            t_kq, t_v = pending[h]

            # two 1024-query halves per head: AV accumulators use only 2 PSUM
            # banks so the QK wave pool can run 3-deep (ACT and DVE exps for
            # consecutive k-blocks overlap instead of serializing)
            for qg in range(2):
                p_av = [
                    mp.tile([DA, 512], F32, tag=f"av{j}", name=f"av{h}_{qg}_{j}")
                    for j in range(2)
                ]

                def emit_av(kb, t_pt):
                    for j in range(2):
                        nc.tensor.matmul(
                            p_av[j][:],
                            t_v[:, kb, :],
                            t_pt[:, j * 512 : (j + 1) * 512],
                            start=(kb == 0),
                            stop=(kb == KB - 1),
                        )

                # software pipeline, 1 k-block deep: AV(kb-1) issues after
                # QK(kb)+exp(kb), so each exp gets a full QK-pair + AV-pair
                # (~850ns) of PE time before its dependent AV can stall the
                # in-order tensor queue (2-deep measured worse: two trailing
                # AV pairs per sweep run past the last QK and stall on exps)
                hist = []
                for kb in range(KB):
                    pw = qkp.tile([128, 1024], F32, tag="wave")
                    for j in range(2):
                        qc = qg * 2 + j
                        nc.tensor.matmul(
                            pw[:, j * 512 : (j + 1) * 512],
                            t_kq[:, _kt_col(kb) : _kt_col(kb) + 128],
                            t_kq[:, _q_col(qc) : _q_col(qc) + 512],
                            start=True,
                            stop=True,
                        )
                    t_pt = ptpool.tile(
                        [128, 1024], BF16, tag="pt", name=f"pt{h}_{qg}_{kb}"
                    )
                    # every exp splits at the j-chunk boundary: ACT and
                    # DVE work the two halves concurrently, freeing the
                    # wave bank ~500ns earlier (deeper effective pipeline)
                    nc.scalar.activation(
                        t_pt[:, 0:512],
                        pw[:, 0:512],
                        mybir.ActivationFunctionType.Exp,
                        bias=0.0,
                        scale=1.0,
                    )
                    nc.vector.tensor_scalar(
                        t_pt[:, 512:1024].bitcast(U16),
                        pw[:, 512:1024],
                        EXP_A,
                        EXP_B,
                        AluOpType.mult,
                        AluOpType.add,
                    )
                    hist.append((kb, t_pt))
                    if len(hist) > 1:
                        emit_av(*hist.pop(0))
                for it in hist:
                    emit_av(*it)

                # drain accumulators: outT rows 0..63 = unnormalized out^T,
                # row 64 = softmax denominator; host divides + transposes.
                # Copies split ACT/DVE; the DMA issues from the idle gpsimd
                # queue. The very last drain is latency-critical (nothing
                # left to overlap with) so each 256-col quarter is copied on
                # its own engine and the two DMAs issue concurrently from
                # the by-then-idle sync and scalar HWDGE queues.
                t_outT = wkpool.tile([DA, 1024], F32, tag="outT")
                last = h == HPC - 1 and qg == 1
                if last:
                    nc.scalar.activation(
                        t_outT[:, 0:256],
                        p_av[0][:, 0:256],
                        mybir.ActivationFunctionType.Copy,
                    )
                    nc.vector.tensor_copy(t_outT[:, 256:512], p_av[0][:, 256:512])
                    nc.sync.dma_start(
                        out=d_out[h][:, qg * 1024 : qg * 1024 + 512],
                        in_=t_outT[:, 0:512],
                    )
                    nc.scalar.activation(
                        t_outT[:, 512:768],
                        p_av[1][:, 0:256],
                        mybir.ActivationFunctionType.Copy,
                    )
                    nc.vector.tensor_copy(t_outT[:, 768:1024], p_av[1][:, 256:512])
                    # j1 DMA also on sync: its DIRECT2D runs while the j1
                    # copies finish (gpsimd dispatches ~0.8us late here and
                    # scalar's HWDGE measured ~1.4us)
                    nc.sync.dma_start(
                        out=d_out[h][:, qg * 1024 + 512 : qg * 1024 + 1024],
                        in_=t_outT[:, 512:1024],
                    )
                else:
                    nc.scalar.activation(
                        t_outT[:, 0:512],
                        p_av[0][:],
                        mybir.ActivationFunctionType.Copy,
                    )
                    nc.vector.tensor_copy(t_outT[:, 512:1024], p_av[1][:])
                    nc.gpsimd.dma_start(
                        out=d_out[h][:, qg * 1024 : (qg + 1) * 1024],
                        in_=t_outT[:],
                    )

            # prefetch after the drains so output DMAs never queue behind
            # this load's buffer-free wait on the gpsimd queue
            while len(pending) < min(HPC, h + 3):
                pending.append(load_head(len(pending)))

    nc.compile()
    return nc


def kernel(
    q: np.ndarray,
    k: np.ndarray,
    v: np.ndarray,
    scale_factor: np.ndarray,
    inv_scale: np.ndarray,
) -> np.ndarray:
    global LAST_RESULT, _CACHED_NC

    q = np.asarray(q, np.float32)
    k = np.asarray(k, np.float32)
    v = np.asarray(v, np.float32)
    scale_factor = np.asarray(scale_factor, np.float32)
    inv_scale = np.asarray(inv_scale, np.float32)

    # host-side input marshaling
    r = 1.0 / (scale_factor * inv_scale[..., None])  # [B,H,S]
    qs = q * r[..., None]  # [B,H,S,D]
    mhat = 5.0 * np.sqrt((qs.astype(np.float64) ** 2).sum(-1)).astype(np.float32)
    q_aug = np.concatenate([qs, -mhat[..., None]], axis=-1)  # [B,H,S,DA]
    k_aug = np.concatenate([k, np.ones((B, H, S, 1), np.float32)], axis=-1)
    v_aug = np.concatenate([v, np.ones((B, H, S, 1), np.float32)], axis=-1)

    qT = q_aug.transpose(0, 1, 3, 2).astype(np.float16)  # [B,H,DA,S]
    kT = k_aug.transpose(0, 1, 3, 2).astype(np.float16)
    # packed first-need layout (see _build_nc): kT blk0-1 | qT c0-1 |
    # kT blk2-15 | qT c2-3
    kq = np.concatenate(
        [kT[..., 0:256], qT[..., 0:1024], kT[..., 256:2048], qT[..., 1024:2048]],
        axis=-1,
    )
    # [B,H,S,DA] -> [B,H,KB,128,DA] -> [B,H,128,KB,DA]
    v16 = np.ascontiguousarray(
        v_aug.reshape(B, H, KB, 128, DA).transpose(0, 1, 3, 2, 4)
    ).astype(np.float16)

    kq = np.ascontiguousarray(kq).reshape(N_CORES, HPC, DA, 2 * S)
    v16 = v16.reshape(N_CORES, HPC, 128, KB, DA)

    _maybe_install_ntff_hook()
    if _CACHED_NC is None:
        _CACHED_NC = _build_nc()
    nc = _CACHED_NC

    in_maps = [{"kq": kq[c], "v": v16[c]} for c in range(N_CORES)]
    res = run_bass_kernel_spmd(nc, in_maps, list(range(N_CORES)))
    LAST_RESULT = res
    outT = np.stack([res.results[c]["outT"] for c in range(N_CORES)])  # [8,HPC,DA,S]
    out = outT[:, :, :D, :] / outT[:, :, D : D + 1, :]
    return (
        np.ascontiguousarray(out.transpose(0, 1, 3, 2))
        .reshape(B, H, S, D)
        .astype(np.float32)
    )


# revision 38
# speedup vs baseline: 1.0037x; 1.0020x over previous
"""Trainium2 Bass kernel: scaled-softmax attention, B=4 H=16 S=2048 D=64.

Sharding: batch*heads (64) across 8 NeuronCores, 8 heads per core.

Per head, on-device, processed in two 1024-query halves (qg):
  for each k-block kb (128 keys):
    S^T[kb] = kT_aug[kb] @ qT_aug    (fp16 x fp16 - same 11-bit mantissa as
              fp32r but keeps the whole matmul stream in 16-bit mode: no
              ~195ns PE dtype-switch penalty, and every matmul counts as HAM
              activity so the 2.4GHz clock never re-throttles. Contraction
              65 = 64 dims + fused row subtracting the per-query bound m_hat)
    P^T[kb] = exp(S^T[kb])           every wave split at the j-chunk
              boundary across both engines working concurrently (frees the
              PSUM wave bank ~500ns earlier -> deeper effective pipeline):
                - ACT (scalar): table exp on cols [0:512]
                - DVE (vector): Schraudolph bit-trick exp on [512:1024] -
                  bf16 bit patterns via round-to-nearest f32->uint16 convert
                  with saturation (deep-negative -> +0.0)
    av[qc] += [v|1][kb] @ P^T[kb]    (fp16 x bf16, K=128 accumulated in PSUM;
              the ones-column makes row 64 the softmax denominator; issued
              one k-block behind the QK/exp front so a late exp never stalls
              the in-order tensor queue)
  outT (rows 0..63 = unnormalized out^T, row 64 = denominator) -> HBM
  (drain copies split ACT/DVE, DMA from the idle gpsimd software-DGE).

DMA-issue plumbing (each dma_start costs ~0.6-1us of issuing-queue time):
kT and qT are packed host-side into one dram tensor per head in first-need
column order, so each head is a single sync-queue DMA (head 0: two pieces,
the first covering the initial ~2us of compute); the otherwise-idle gpsimd
software-DGE issues v + all output drains so the sync queue reaches head
0's transfers immediately and never backs up behind output traffic. Later
heads prefetch only after the current head's body so their bulk transfers
cannot crowd the latency-critical first pieces off the DMA rings.

bf16 warm-up matmuls on const data run before the first real matmul to trip
the PE HAM un-throttle (1.2 -> 2.4 GHz) while the first input DMAs land.

Host (numpy) does input/output marshaling: q scaled by 1/(scale_factor*
inv_scale), m_hat = 5*||q_scaled||, transpose/augment/fp16 rounding on the
way in; per-query divide by the denominator row + transpose on the way out.
"""

import os
import sys

sys.path.insert(0, "/opt/trn_rl_repo")

from contextlib import ExitStack

import numpy as np

import concourse.bass as bass
import concourse.tile as tile
from concourse import bacc, mybir
from concourse.alu_op_type import AluOpType
from concourse.bass_utils import run_bass_kernel_spmd

B, H, S, D = 4, 16, 2048, 64
N_CORES = 8
HPC = (B * H) // N_CORES  # heads per core
KB = S // 128  # 16 k-blocks
DA = D + 1  # augmented contraction dim (65)

F32 = mybir.dt.float32
F32R = mybir.dt.float32r
BF16 = mybir.dt.bfloat16
F16 = mybir.dt.float16
U16 = mybir.dt.uint16

# Schraudolph bf16 exp: bits16 = round(x * EXP_A + EXP_B); rel err ~ +-3.1%
EXP_A = 184.6650390625  # 2^7 / ln(2)
EXP_B = 16250.25  # 127*2^7 with minimax centering

N_WARM = 3  # bf16 HAM warm-up pairs: bridge until the first input DMA lands

LAST_RESULT = None
_CACHED_NC = None


def _maybe_install_ntff_hook():
    """BASS_TRACE=1 needs antenv.axon_hooks, absent from this image; inject it."""
    if not os.environ.get("BASS_TRACE") or "antenv.axon_hooks" in sys.modules:
        return
    try:
        import types

        import antenv
        from trn_agent_boot.trn_boot import _ntff_profile_via_ctypes

        mod = types.ModuleType("antenv.axon_hooks")
        mod._hook = None
        mod.set_axon_ntff_profile_hook = lambda h: setattr(mod, "_hook", h)
        mod.get_axon_ntff_profile_hook = lambda: mod._hook
        sys.modules["antenv.axon_hooks"] = mod
        antenv.axon_hooks = mod
        mod.set_axon_ntff_profile_hook(
            _ntff_profile_via_ctypes("/opt/axon/libaxon_pjrt.so")
        )
    except Exception:
        os.environ["BASS_NEVER_TRACE"] = "1"


def _kt_col(kb):
    """Column of kT block kb inside the packed kq layout."""
    return kb * 128 if kb < 2 else 1280 + (kb - 2) * 128


def _q_col(qc):
    """Column of qT chunk qc (512 queries) inside the packed kq layout."""
    return 256 + qc * 512 if qc < 2 else 3072 + (qc - 2) * 512


def _build_nc():
    nc = bacc.Bacc("TRN2", target_bir_lowering=False, debug=False)

    # kq packs kT and qT per head in first-need column order:
    #   [0:256]=kT blk0-1 | [256:1280]=qT chunk0-1 | [1280:3072]=kT blk2-15
    #   | [3072:4096]=qT chunk2-3
    # so head 0 streams in as two contiguous pieces (piece A = cols 0:1280
    # covers the whole first ~2us of compute) and later heads as one DMA —
    # each dma_start costs ~0.8us of issuing-queue time, so fewer, bigger
    # issues in need order is what gets the PE fed early.
    d_kq = nc.dram_tensor("kq", [HPC, DA, 2 * S], F16, kind="ExternalInput").ap()
    d_v = nc.dram_tensor("v", [HPC, 128, KB, DA], F16, kind="ExternalInput").ap()
    d_out = nc.dram_tensor("outT", [HPC, DA, S], F32, kind="ExternalOutput").ap()

    with tile.TileContext(nc) as tc, ExitStack() as ctx:
        cpool = ctx.enter_context(tc.tile_pool(name="consts", bufs=1))
        inpool = ctx.enter_context(tc.tile_pool(name="in", bufs=3))
        ptpool = ctx.enter_context(tc.tile_pool(name="pt", bufs=4))
        wkpool = ctx.enter_context(tc.tile_pool(name="wk", bufs=3))
        qkp = ctx.enter_context(tc.tile_pool(name="qkp", bufs=3, space="PSUM"))
        mp = ctx.enter_context(tc.tile_pool(name="mp", bufs=1, space="PSUM"))

        # warm-up consts: memsets on DVE, whose queue comes up first and has
        # nothing else to do, so the HAM warm-up matmuls can issue the moment
        # the tensor queue starts
        t_junkw = cpool.tile([128, 128], BF16)
        nc.vector.memset(t_junkw[:], 0.0)
        t_junk = cpool.tile([128, 512], BF16)
        nc.vector.memset(t_junk[:], 0.0)
        t_warm = cpool.tile([1, 1], F32)
        # trigger the ACT exp table load while input DMAs run
        nc.scalar.activation(
            t_warm[:], t_junkw[0:1, 0:1], mybir.ActivationFunctionType.Exp
        )

        def load_head(h):
            t_kq = inpool.tile([DA, 2 * S], F16, tag="kq", name=f"kq{h}")
            t_v = inpool.tile([128, KB, DA], F16, tag="v", name=f"v{h}")
            if h == 0:
                # split so the first piece (everything the first ~2us of
                # compute touches) lands as early as possible
                nc.sync.dma_start(out=t_kq[:, 0:1280], in_=d_kq[h][:, 0:1280])
                nc.sync.dma_start(out=t_kq[:, 1280:4096], in_=d_kq[h][:, 1280:4096])
            else:
                nc.sync.dma_start(out=t_kq[:], in_=d_kq[h])
            nc.gpsimd.dma_start(out=t_v[:], in_=d_v[h])
            return t_kq, t_v

        # head-0 input DMAs lead the sync queue so the rings start
        # immediately; later heads are requested only after head 0's body is
        # emitted so their bulk transfers cannot crowd head 0's
        # latency-critical pieces off the DMA rings
        pending = [load_head(0)]

        # HAM pre-warm: bf16 matmuls on const data keep the PE busy while the
        # first input DMA lands so the clock un-throttles before real work
        # (fp32-mode matmuls do not register as HAM activity). Two tiny
        # bootstrap matmuls gated only on the small memset start the
        # activity clock ~0.8us earlier; then one tile with alternating
        # halves, WAW-chained back-to-back with no pool rotation gaps.
        pwarm = qkp.tile([128, 1024], F32, tag="wave", name="pwarm")
        for w in range(2):
            nc.tensor.matmul(
                pwarm[:, w * 128 : (w + 1) * 128],
                t_junkw[:],
                t_junkw[:],
                start=True,
                stop=True,
            )
        for w in range(2 * N_WARM):
            nc.tensor.matmul(
                pwarm[:, (w % 2) * 512 : (w % 2 + 1) * 512],
                t_junkw[:],
                t_junk[:],
                start=True,
                stop=True,
            )

        for h in range(HPC):
            t_kq, t_v = pending[h]

            # two 1024-query halves per head: AV accumulators use only 2 PSUM
            # banks so the QK wave pool can run 3-deep (ACT and DVE exps for
            # consecutive k-blocks overlap instead of serializing)
            for qg in range(2):
                p_av = [
                    mp.tile([DA, 512], F32, tag=f"av{j}", name=f"av{h}_{qg}_{j}")
                    for j in range(2)
                ]

                def emit_av(kb, t_pt):
                    for j in range(2):
                        nc.tensor.matmul(
                            p_av[j][:],
                            t_v[:, kb, :],
                            t_pt[:, j * 512 : (j + 1) * 512],
                            start=(kb == 0),
                            stop=(kb == KB - 1),
                        )

                # software pipeline, 1 k-block deep: AV(kb-1) issues after
                # QK(kb)+exp(kb), so each exp gets a full QK-pair + AV-pair
                # (~850ns) of PE time before its dependent AV can stall the
                # in-order tensor queue (2-deep measured worse: two trailing
                # AV pairs per sweep run past the last QK and stall on exps)
                hist = []
                for kb in range(KB):
                    pw = qkp.tile([128, 1024], F32, tag="wave")
                    for j in range(2):
                        qc = qg * 2 + j
                        nc.tensor.matmul(
                            pw[:, j * 512 : (j + 1) * 512],
                            t_kq[:, _kt_col(kb) : _kt_col(kb) + 128],
                            t_kq[:, _q_col(qc) : _q_col(qc) + 512],
                            start=True,
                            stop=True,
                        )
                    t_pt = ptpool.tile(
                        [128, 1024], BF16, tag="pt", name=f"pt{h}_{qg}_{kb}"
                    )
                    # every exp splits at the j-chunk boundary: ACT and
                    # DVE work the two halves concurrently, freeing the
                    # wave bank ~500ns earlier (deeper effective pipeline)
                    nc.scalar.activation(
                        t_pt[:, 0:512],
                        pw[:, 0:512],
                        mybir.ActivationFunctionType.Exp,
                        bias=0.0,
                        scale=1.0,
                    )
                    nc.vector.tensor_scalar(
                        t_pt[:, 512:1024].bitcast(U16),
                        pw[:, 512:1024],
                        EXP_A,
                        EXP_B,
                        AluOpType.mult,
                        AluOpType.add,
                    )
                    hist.append((kb, t_pt))
                    if len(hist) > 1:
                        emit_av(*hist.pop(0))
                for it in hist:
                    emit_av(*it)

                # drain accumulators: outT rows 0..63 = unnormalized out^T,
                # row 64 = softmax denominator; host divides + transposes.
                # Copies split ACT/DVE; the DMA issues from the idle gpsimd
                # queue. The very last drain is latency-critical (nothing
                # left to overlap with) so each 256-col quarter is copied on
                # its own engine and the two DMAs issue concurrently from
                # the by-then-idle sync and scalar HWDGE queues.
                t_outT = wkpool.tile([DA, 1024], F32, tag="outT")
                last = h == HPC - 1 and qg == 1
                if last:
                    nc.scalar.activation(
                        t_outT[:, 0:256],
                        p_av[0][:, 0:256],
                        mybir.ActivationFunctionType.Copy,
                    )
                    nc.vector.tensor_copy(t_outT[:, 256:512], p_av[0][:, 256:512])
                    nc.sync.dma_start(
                        out=d_out[h][:, qg * 1024 : qg * 1024 + 512],
                        in_=t_outT[:, 0:512],
                    )
                    nc.scalar.activation(
                        t_outT[:, 512:768],
                        p_av[1][:, 0:256],
                        mybir.ActivationFunctionType.Copy,
                    )
                    nc.vector.tensor_copy(t_outT[:, 768:1024], p_av[1][:, 256:512])
                    # j1 DMA also on sync: its DIRECT2D runs while the j1
                    # copies finish (gpsimd dispatches ~0.8us late here and
                    # scalar's HWDGE measured ~1.4us)
                    nc.sync.dma_start(
                        out=d_out[h][:, qg * 1024 + 512 : qg * 1024 + 1024],
                        in_=t_outT[:, 512:1024],
                    )
                else:
                    nc.scalar.activation(
                        t_outT[:, 0:512],
                        p_av[0][:],
                        mybir.ActivationFunctionType.Copy,
                    )
                    nc.vector.tensor_copy(t_outT[:, 512:1024], p_av[1][:])
                    nc.gpsimd.dma_start(
                        out=d_out[h][:, qg * 1024 : (qg + 1) * 1024],
                        in_=t_outT[:],
                    )

            # prefetch after the drains so output DMAs never queue behind
            # this load's buffer-free wait on the gpsimd queue
            while len(pending) < min(HPC, h + 3):
                pending.append(load_head(len(pending)))

    nc.compile()
    return nc


def kernel(
    q: np.ndarray,
    k: np.ndarray,
    v: np.ndarray,
    scale_factor: np.ndarray,
    inv_scale: np.ndarray,
) -> np.ndarray:
    global LAST_RESULT, _CACHED_NC

    q = np.asarray(q, np.float32)
    k = np.asarray(k, np.float32)
    v = np.asarray(v, np.float32)
    scale_factor = np.asarray(scale_factor, np.float32)
    inv_scale = np.asarray(inv_scale, np.float32)

    # host-side input marshaling
    r = 1.0 / (scale_factor * inv_scale[..., None])  # [B,H,S]
    qs = q * r[..., None]  # [B,H,S,D]
    mhat = 5.0 * np.sqrt((qs.astype(np.float64) ** 2).sum(-1)).astype(np.float32)
    q_aug = np.concatenate([qs, -mhat[..., None]], axis=-1)  # [B,H,S,DA]
    k_aug = np.concatenate([k, np.ones((B, H, S, 1), np.float32)], axis=-1)
    v_aug = np.concatenate([v, np.ones((B, H, S, 1), np.float32)], axis=-1)

    qT = q_aug.transpose(0, 1, 3, 2).astype(np.float16)  # [B,H,DA,S]
    kT = k_aug.transpose(0, 1, 3, 2).astype(np.float16)
    # packed first-need layout (see _build_nc): kT blk0-1 | qT c0-1 |
    # kT blk2-15 | qT c2-3
    kq = np.concatenate(
        [kT[..., 0:256], qT[..., 0:1024], kT[..., 256:2048], qT[..., 1024:2048]],
        axis=-1,
    )
    # [B,H,S,DA] -> [B,H,KB,128,DA] -> [B,H,128,KB,DA]
    v16 = np.ascontiguousarray(
        v_aug.reshape(B, H, KB, 128, DA).transpose(0, 1, 3, 2, 4)
    ).astype(np.float16)

    kq = np.ascontiguousarray(kq).reshape(N_CORES, HPC, DA, 2 * S)
    v16 = v16.reshape(N_CORES, HPC, 128, KB, DA)

    _maybe_install_ntff_hook()
    if _CACHED_NC is None:
        _CACHED_NC = _build_nc()
    nc = _CACHED_NC

    in_maps = [{"kq": kq[c], "v": v16[c]} for c in range(N_CORES)]
    res = run_bass_kernel_spmd(nc, in_maps, list(range(N_CORES)))
    LAST_RESULT = res
    outT = np.stack([res.results[c]["outT"] for c in range(N_CORES)])  # [8,HPC,DA,S]
    out = outT[:, :, :D, :] / outT[:, :, D : D + 1, :]
    return (
        np.ascontiguousarray(out.transpose(0, 1, 3, 2))
        .reshape(B, H, S, D)
        .astype(np.float32)
    )
